# revision 29
# baseline (speedup 1.0000x reference)
import sys

sys.path.insert(0, "/opt/trn_rl_repo")

import numpy as np

import concourse.bass as bass
import concourse.tile as tile
from concourse import bacc, mybir
from concourse.bass_utils import run_bass_kernel_spmd

F32 = mybir.dt.float32
F32R = mybir.dt.float32r
AF = mybir.ActivationFunctionType

BATCH = 2
SEQ = 2048
D = 1024
NHEADS = 16
DK = 64
HPC = 4          # heads per core
NCORES = 8
THETA = 10000.0
EPS = 1e-8
NEG = -30000.0
CHUNK = 512
NCH = SEQ // CHUNK   # 4 chunks of queries
NBLK = SEQ // 128    # 16 key blocks


def _build_nc():
    nc = bacc.Bacc("TRN2", target_bir_lowering=False)
    XT = nc.declare_dram_parameter("XT", [128, 8, SEQ], F32, isOutput=False)
    WT = nc.declare_dram_parameter("WT", [128, 8, 768], F32, isOutput=False)
    COS = nc.declare_dram_parameter("COS", [128, SEQ], F32, isOutput=False)
    SIN = nc.declare_dram_parameter("SIN", [128, SEQ], F32, isOutput=False)
    WOT = nc.declare_dram_parameter("WOT", [128, 2, D], F32, isOutput=False)
    MASKB = nc.declare_dram_parameter("MASKB", [128, 896], F32, isOutput=False)
    INDT = nc.declare_dram_parameter("INDT", [128, 4], F32, isOutput=False)
    I2Q = nc.declare_dram_parameter("I2Q", [4, 128], F32, isOutput=False)
    I2K = nc.declare_dram_parameter("I2K", [4, 128], F32, isOutput=False)
    ID = nc.declare_dram_parameter("ID", [128, 128], F32, isOutput=False)
    OUT = nc.declare_dram_parameter("OUT", [SEQ, D], F32, isOutput=True)

    with tile.TileContext(nc) as tc:
        with (
            nc.allow_low_precision(reason="float32r tags carry fp32 bits"),
            tc.tile_pool(name="cst", bufs=1) as cst,
            tc.tile_pool(name="xtp", bufs=2) as xtp,
            tc.tile_pool(name="tmp", bufs=8) as tmp,
            tc.tile_pool(name="expp", bufs=3) as expp,
            tc.tile_pool(name="bcp", bufs=2) as bcp,
            tc.tile_pool(name="ocp", bufs=2) as ocp,
            tc.tile_pool(name="ps", bufs=6, space="PSUM") as ps,
        ):
            wt_sb = cst.tile([128, 8, 768], F32, tag="wt")
            cos_sb = cst.tile([128, SEQ], F32, tag="cos")
            sin_sb = cst.tile([128, SEQ], F32, tag="sin")
            wot_sb = cst.tile([128, 2, D], F32, tag="wot")
            mask_sb = cst.tile([128, 896], F32, tag="mask")
            indt_sb = cst.tile([128, 4], F32, tag="indt")
            i2q_sb = cst.tile([4, 128], F32, tag="i2q")
            i2k_sb = cst.tile([4, 128], F32, tag="i2k")
            id_sb = cst.tile([128, 128], F32, tag="id")
            q_sb = cst.tile([128, 2, SEQ], F32, tag="q")
            k_sb = cst.tile([128, 2, SEQ], F32, tag="k")
            v_sb = cst.tile([128, NBLK, 260], F32, tag="v")
            ot_sb = cst.tile([128, 2, SEQ], F32, tag="ot")

            nc.sync.dma_start(out=wt_sb[:].bitcast(F32R), in_=WT[:].bitcast(F32R))
            nc.sync.dma_start(out=cos_sb[:], in_=COS[:])
            nc.sync.dma_start(out=sin_sb[:], in_=SIN[:])
            nc.sync.dma_start(out=wot_sb[:].bitcast(F32R), in_=WOT[:].bitcast(F32R))
            nc.sync.dma_start(out=mask_sb[:].bitcast(F32R), in_=MASKB[:].bitcast(F32R))
            nc.sync.dma_start(out=indt_sb[:].bitcast(F32R), in_=INDT[:].bitcast(F32R))
            nc.sync.dma_start(out=i2q_sb[:].bitcast(F32R), in_=I2Q[:].bitcast(F32R))
            nc.sync.dma_start(out=i2k_sb[:].bitcast(F32R), in_=I2K[:].bitcast(F32R))
            nc.sync.dma_start(out=id_sb[:].bitcast(F32R), in_=ID[:].bitcast(F32R))

            # ones columns for the denominator trick (data cols overwritten below)
            # memset can't emit f32r; memset f32 then round via DVE self-copy
            nc.vector.memset(v_sb[:], 1.0)
            nc.vector.tensor_copy(v_sb[:].bitcast(F32R), v_sb[:])
            ones1 = cst.tile([1, 64], F32, tag="ones1")
            nc.vector.memset(ones1[:], 1.0)
            nc.vector.tensor_copy(ones1[:].bitcast(F32R), ones1[:])

            def emit_proj(c):
                c0 = c * CHUNK
                xt_t = xtp.tile([128, 8, CHUNK], F32, tag="xt", name=f"xt_{c}")
                nc.sync.dma_start(out=xt_t[:].bitcast(F32R),
                                  in_=XT[:, :, c0:c0 + CHUNK].bitcast(F32R))

                # ---- Q/K projection + L2 norm + RoPE ----
                for qk in range(2):
                    qoff = 256 * qk
                    dst = q_sb if qk == 0 else k_sb
                    i2 = i2q_sb if qk == 0 else i2k_sb

                    pA = ps.tile([128, CHUNK], F32, tag="mm", name=f"pA_{qk}_{c}")
                    for di in range(8):
                        nc.tensor.matmul(
                            pA,
                            lhsT=wt_sb[:, di, qoff:qoff + 128].bitcast(F32R),
                            rhs=xt_t[:, di, :].bitcast(F32R),
                            start=(di == 0), stop=(di == 7),
                        )
                    pB = ps.tile([128, CHUNK], F32, tag="mm", name=f"pB_{qk}_{c}")
                    for di in range(8):
                        nc.tensor.matmul(
                            pB,
                            lhsT=wt_sb[:, di, qoff + 128:qoff + 256].bitcast(F32R),
                            rhs=xt_t[:, di, :].bitcast(F32R),
                            start=(di == 0), stop=(di == 7),
                        )

                    sqA = tmp.tile([128, CHUNK], F32, tag="t", name=f"sqA_{qk}_{c}")
                    nc.scalar.activation(sqA[:], pA[:], AF.Square)
                    sqB = tmp.tile([128, CHUNK], F32, tag="t", name=f"sqB_{qk}_{c}")
                    nc.scalar.activation(sqB[:], pB[:], AF.Square)
                    ssum = tmp.tile([128, CHUNK], F32, tag="t", name=f"ssum_{qk}_{c}")
                    nc.vector.tensor_add(ssum[:].bitcast(F32R), sqA[:], sqB[:])

                    n2 = ps.tile([4, CHUNK], F32, tag="mm", name=f"n2_{qk}_{c}")
                    nc.tensor.matmul(
                        n2, lhsT=indt_sb[:].bitcast(F32R),
                        rhs=ssum[:].bitcast(F32R), start=True, stop=True,
                    )
                    nrm = tmp.tile([4, CHUNK], F32, tag="t", name=f"nrm_{qk}_{c}")
                    nc.scalar.activation(nrm[:], n2[:], AF.Sqrt)
                    nc.vector.tensor_scalar_add(nrm[:], nrm[:], EPS)
                    nrmr = tmp.tile([4, CHUNK], F32, tag="t", name=f"nrmr_{qk}_{c}")
                    nc.vector.reciprocal(nrmr[:].bitcast(F32R), nrm[:])

                    rbp = ps.tile([128, CHUNK], F32, tag="mm", name=f"rbp_{qk}_{c}")
                    nc.tensor.matmul(
                        rbp, lhsT=i2[:].bitcast(F32R),
                        rhs=nrmr[:].bitcast(F32R), start=True, stop=True,
                    )
                    rb = tmp.tile([128, CHUNK], F32, tag="t", name=f"rb_{qk}_{c}")
                    nc.vector.tensor_copy(rb[:], rbp[:])

                    An = tmp.tile([128, CHUNK], F32, tag="t", name=f"An_{qk}_{c}")
                    nc.vector.tensor_mul(An[:], pA[:], rb[:])
                    Bn = tmp.tile([128, CHUNK], F32, tag="t", name=f"Bn_{qk}_{c}")
                    nc.vector.tensor_mul(Bn[:], pB[:], rb[:])

                    cs = cos_sb[:, c0:c0 + CHUNK]
                    sn = sin_sb[:, c0:c0 + CHUNK]
                    tac = tmp.tile([128, CHUNK], F32, tag="t", name=f"tac_{qk}_{c}")
                    nc.vector.tensor_mul(tac[:], An[:], cs)
                    tbs = tmp.tile([128, CHUNK], F32, tag="t", name=f"tbs_{qk}_{c}")
                    nc.vector.tensor_mul(tbs[:], Bn[:], sn)
                    tas = tmp.tile([128, CHUNK], F32, tag="t", name=f"tas_{qk}_{c}")
                    nc.vector.tensor_mul(tas[:], An[:], sn)
                    tbc = tmp.tile([128, CHUNK], F32, tag="t", name=f"tbc_{qk}_{c}")
                    nc.vector.tensor_mul(tbc[:], Bn[:], cs)

                    for h in range(HPC):
                        po = (h % 2) * 64
                        ti = h // 2
                        hs = 32 * h
                        nc.vector.tensor_sub(
                            dst[po:po + 32, ti, c0:c0 + CHUNK].bitcast(F32R),
                            tac[hs:hs + 32, :], tbs[hs:hs + 32, :])
                        nc.vector.tensor_add(
                            dst[po + 32:po + 64, ti, c0:c0 + CHUNK].bitcast(F32R),
                            tas[hs:hs + 32, :], tbc[hs:hs + 32, :])

                # ---- V projection for this chunk's 4 key blocks ----
                for bb in range(4):
                    nb = 4 * c + bb
                    vps = ps.tile([128, 256], F32, tag="mm", name=f"vps_{nb}")
                    for di in range(8):
                        nc.tensor.matmul(
                            vps,
                            lhsT=xt_t[:, di, bb * 128:bb * 128 + 128].bitcast(F32R),
                            rhs=wt_sb[:, di, 512:768].bitcast(F32R),
                            start=(di == 0), stop=(di == 7),
                        )
                    for h in range(HPC):
                        nc.vector.tensor_copy(
                            v_sb[:, nb, 65 * h:65 * h + 64].bitcast(F32R),
                            vps[:, 64 * h:64 * h + 64])

            def emit_attn(c):
                c0 = c * CHUNK
                njb = 4 * (c + 1)
                for h in range(HPC):
                    po = (h % 2) * 64
                    ti = h // 2
                    av = ps.tile([65, CHUNK], F32, tag="av", bufs=2,
                                 name=f"av_{h}_{c}")
                    def issue_sc(jb):
                        diag = jb >= 4 * c
                        sc = ps.tile([128, CHUNK], F32, tag="mm", name=f"sc_{h}_{c}_{jb}")
                        nc.tensor.matmul(
                            sc,
                            lhsT=k_sb[po:po + 64, ti, jb * 128:jb * 128 + 128].bitcast(F32R),
                            rhs=q_sb[po:po + 64, ti, c0:c0 + CHUNK].bitcast(F32R),
                            start=True, stop=not diag,
                        )
                        if diag:
                            s0 = 384 - 128 * (jb - 4 * c)
                            nc.tensor.matmul(
                                sc,
                                lhsT=id_sb[:].bitcast(F32R),
                                rhs=mask_sb[:, s0:s0 + CHUNK].bitcast(F32R),
                                start=False, stop=True,
                            )
                        return sc

                    # stagger: issue sc for jb+1 before draining jb so PE
                    # keeps ahead of ACT's exp stream
                    cur = issue_sc(0)
                    for jb in range(njb):
                        nxt = issue_sc(jb + 1) if jb + 1 < njb else None
                        ex = expp.tile([128, CHUNK], F32, tag="ex", name=f"ex_{h}_{c}_{jb}")
                        nc.scalar.activation(ex[:].bitcast(F32R), cur[:], AF.Exp)
                        nc.tensor.matmul(
                            av,
                            lhsT=v_sb[:, jb, 65 * h:65 * h + 65].bitcast(F32R),
                            rhs=ex[:].bitcast(F32R),
                            start=(jb == 0), stop=(jb == njb - 1),
                        )
                        cur = nxt

                    srec = bcp.tile([1, CHUNK], F32, tag="srec", name=f"srec_{h}_{c}")
                    nc.vector.reciprocal(srec[:].bitcast(F32R), av[64:65, :])
                    rb2p = ps.tile([64, CHUNK], F32, tag="mm", name=f"rb2p_{h}_{c}")
                    nc.tensor.matmul(
                        rb2p, lhsT=ones1[:].bitcast(F32R),
                        rhs=srec[:].bitcast(F32R), start=True, stop=True,
                    )
                    rb2 = bcp.tile([64, CHUNK], F32, tag="rb2", name=f"rb2_{h}_{c}")
                    nc.vector.tensor_copy(rb2[:], rb2p[:])
                    nc.vector.tensor_mul(
                        ot_sb[po:po + 64, ti, c0:c0 + CHUNK].bitcast(F32R),
                        av[0:64, :], rb2[:])

            def emit_outproj(c):
                for bb in range(4):
                    nb = 4 * c + bb
                    for oc in range(2):
                        wo = ps.tile([128, CHUNK], F32, tag="mm", name=f"wo_{nb}_{oc}")
                        for ti in range(2):
                            nc.tensor.matmul(
                                wo,
                                lhsT=ot_sb[:, ti, nb * 128:nb * 128 + 128].bitcast(F32R),
                                rhs=wot_sb[:, ti, oc * CHUNK:oc * CHUNK + CHUNK].bitcast(F32R),
                                start=(ti == 0), stop=(ti == 1),
                            )
                        ob = ocp.tile([128, CHUNK], F32, tag="ob", name=f"ob_{nb}_{oc}")
                        nc.vector.tensor_copy(ob[:], wo[:])
                        nc.sync.dma_start(
                            out=OUT[nb * 128:nb * 128 + 128, oc * CHUNK:oc * CHUNK + CHUNK],
                            in_=ob[:])

            # software pipeline: chunk c's projections (PE/DVE-heavy) are
            # emitted before chunk c-1's attention (ACT-heavy) so the engine
            # queues overlap instead of phase ping-ponging
            emit_proj(0)
            for c in range(1, NCH):
                emit_proj(c)
                emit_attn(c - 1)
                emit_outproj(c - 1)
            emit_attn(NCH - 1)
            emit_outproj(NCH - 1)
    return nc


_NC = None


def _get_nc():
    global _NC
    if _NC is None:
        _NC = _build_nc()
        _NC.finalize()
    return _NC


def _shared_tables(token_positions):
    freqs = np.arange(0, DK, 2, dtype=np.float64)
    inv_theta = THETA ** (-freqs / DK)                      # [32]
    pos = token_positions.astype(np.float64)
    ang = inv_theta[:, None] * pos[None, :]                 # [32, SEQ]
    cos_t = np.ascontiguousarray(np.tile(np.cos(ang), (4, 1)).astype(np.float32))
    sin_t = np.ascontiguousarray(np.tile(np.sin(ang), (4, 1)).astype(np.float32))

    p_i = np.arange(128)[:, None]
    t_i = np.arange(896)[None, :]
    maskb = np.where(t_i >= p_i + 384, 0.0, NEG).astype(np.float32)

    indt = np.zeros((128, 4), dtype=np.float32)
    for j in range(4):
        indt[32 * j:32 * j + 32, j] = 1.0
    i2k = np.ascontiguousarray(indt.T)
    idm = np.eye(128, dtype=np.float32)
    return cos_t, sin_t, maskb, indt, i2k, idm


def _core_inputs(c, x, W_QKV, W_O, qk_scale, shared):
    cos_t, sin_t, maskb, indt, i2k, idm = shared
    b = c // 4
    a = c % 4
    heads = [4 * a + i for i in range(HPC)]

    qA = [64 * h + 2 * t for h in heads for t in range(32)]
    qB = [64 * h + 2 * t + 1 for h in heads for t in range(32)]
    kA = [1024 + r for r in qA]
    kB = [1024 + r for r in qB]
    vr = [2048 + 64 * h + j for h in heads for j in range(DK)]
    rows = qA + qB + kA + kB + vr
    wt = np.ascontiguousarray(
        W_QKV[rows, :].T.reshape(8, 128, 768).transpose(1, 0, 2))

    vcols = [64 * h + j for h in heads for j in range(DK)]
    wot = np.ascontiguousarray(
        W_O[:, vcols].T.reshape(2, 128, D).transpose(1, 0, 2))

    xt = np.ascontiguousarray(
        x[b].T.reshape(8, 128, SEQ).transpose(1, 0, 2))

    i2q = np.zeros((4, 128), dtype=np.float32)
    for j in range(4):
        i2q[j, 32 * j:32 * j + 32] = np.float32(qk_scale[heads[j]])

    return {
        "XT": xt, "WT": wt, "COS": cos_t, "SIN": sin_t, "WOT": wot,
        "MASKB": maskb, "INDT": indt, "I2Q": i2q, "I2K": i2k, "ID": idm,
    }


def _run(inputs, trace=False):
    x = np.asarray(inputs["x"], dtype=np.float32)
    token_positions = np.asarray(inputs["token_positions"])
    W_QKV = np.asarray(inputs["W_QKV"], dtype=np.float32)
    W_O = np.asarray(inputs["W_O"], dtype=np.float32)
    qk_scale = np.asarray(inputs["qk_scale"], dtype=np.float32)

    shared = _shared_tables(token_positions)
    nc = _get_nc()
    in_maps = [_core_inputs(c, x, W_QKV, W_O, qk_scale, shared)
               for c in range(NCORES)]
    core_ids = list(range(NCORES))
    kw = {}
    if trace:
        kw = dict(trace=True, trace_cores=core_ids)
    res = run_bass_kernel_spmd(nc, in_maps, core_ids, **kw)
    parts = [np.asarray(r["OUT"], dtype=np.float32) for r in res.results]
    out = np.stack([
        parts[0] + parts[1] + parts[2] + parts[3],
        parts[4] + parts[5] + parts[6] + parts[7],
    ]).astype(np.float32)
    return out, getattr(res, "exec_time_ns", None)


def kernel(**inputs):
    return _run(inputs, trace=False)[0]


def estimate_time_ns():
    from concourse.timeline_sim import TimelineSim
    ts = TimelineSim(_get_nc(), trace=False, no_exec=True)
    return ts.simulate()


def kernel_timed(**inputs):
    out, _ = _run(inputs, trace=False)
    return out, estimate_time_ns()
